# revision 1
# baseline (speedup 1.0000x reference)
"""CrissCross (axial) attention, data-parallel over batch on 8 NeuronCores.

Shapes (hardcoded from the problem spec):
  x     [16, 64, 128, 128] f32
  Wq,Wk [8, 64]   bq,bk [8]
  Wv    [64, 64]  bv [64]
  gamma [1]

Sharding: pure data parallel — batch 16 split 2-per-core across 8 cores;
weights replicated. Each core computes its own axial attention; results
are gathered back to the full [16, 64, 128, 128] output.
"""

import jax
import jax.numpy as jnp
import numpy as np

B, C, H, W = 16, 64, 128, 128
N_CORES = 8


def _criss_cross_local(x, Wq, bq, Wk, bk, Wv, bv, gamma):
    # x: [B/8, C, H, W] on one core
    q = jnp.einsum("bchw,oc->bohw", x, Wq) + bq[None, :, None, None]
    k = jnp.einsum("bchw,oc->bohw", x, Wk) + bk[None, :, None, None]
    v = jnp.einsum("bchw,oc->bohw", x, Wv) + bv[None, :, None, None]

    h = x.shape[2]
    # bf16 operands with fp32 accumulation: TensorE runs bf16 at full rate
    # (fp32 matmul is 4 cycles/row); softmax + final combine stay fp32.
    qb = q.astype(jnp.bfloat16)
    kb = k.astype(jnp.bfloat16)
    vb = v.astype(jnp.bfloat16)
    f32 = jnp.float32
    energy_H = jnp.einsum("bchw,bciw->bhwi", qb, kb, preferred_element_type=f32)
    diag = jnp.eye(h, dtype=bool)[None, :, None, :]
    energy_H = jnp.where(diag, -jnp.inf, energy_H)
    energy_W = jnp.einsum("bchw,bchj->bhwj", qb, kb, preferred_element_type=f32)

    # joint softmax over the concatenated H+W key axis, computed without
    # materializing the [b,H,W,H+W] concat: subtract the joint max, exp each
    # part, normalize by the joint sum.
    m = jnp.maximum(
        energy_H.max(axis=3, keepdims=True), energy_W.max(axis=3, keepdims=True)
    )
    p_H = jnp.exp(energy_H - m)
    p_W = jnp.exp(energy_W - m)
    z = p_H.sum(axis=3, keepdims=True) + p_W.sum(axis=3, keepdims=True)  # [b,H,W,1]

    pHb = p_H.astype(jnp.bfloat16)
    pWb = p_W.astype(jnp.bfloat16)
    out_H = jnp.einsum("bhwi,bciw->bchw", pHb, vb, preferred_element_type=f32)
    out_W = jnp.einsum("bhwj,bchj->bchw", pWb, vb, preferred_element_type=f32)
    rz = (1.0 / z)[:, :, :, 0][:, None]  # [b,1,H,W]
    return gamma[0] * ((out_H + out_W) * rz) + x


_pmapped = None


def _get_pmapped():
    global _pmapped
    if _pmapped is None:
        _pmapped = jax.pmap(
            _criss_cross_local,
            axis_name="i",
            in_axes=(0, None, None, None, None, None, None, None),
            devices=jax.devices()[:N_CORES],
        )
    return _pmapped


def kernel(x, Wq, bq, Wk, bk, Wv, bv, gamma):
    x = np.asarray(x, dtype=np.float32)
    xs = x.reshape(N_CORES, B // N_CORES, C, H, W)
    fn = _get_pmapped()
    out = fn(
        xs,
        jnp.asarray(Wq, jnp.float32),
        jnp.asarray(bq, jnp.float32),
        jnp.asarray(Wk, jnp.float32),
        jnp.asarray(bk, jnp.float32),
        jnp.asarray(Wv, jnp.float32),
        jnp.asarray(bv, jnp.float32),
        jnp.asarray(gamma, jnp.float32),
    )
    out = np.asarray(out).reshape(B, C, H, W).astype(np.float32)
    return out

